# revision 12
# baseline (speedup 1.0000x reference)
"""Multi-head attention (N=4, L=1024, E=1024, H=16, D=64) on 8 trn2 NeuronCores.

Sharding: core c = (batch n = c//2, head-group g = c%2); each core owns 8 heads
of one batch. Projections + attention + a partial output projection run on
device; the host sums the two per-batch partials and adds the output bias.

Device-side schedule (all matmuls bf16):
  - host pre-transposes x and weights; wq/bq pre-scaled by 1/sqrt(E) so the
    softmax exp needs no scale stage;
  - q,k projections produce qT/kT in [head_dim, pos] layout (head pair 2m at
    partitions 0-63, 2m+1 at 64-127), v in natural [pos, head_dim|1] layout;
  - scores run as 64-row PE row-tiles: the two heads of a pair stream
    concurrently through disjoint halves of the PE array (T0/T8);
  - exp on ScalarE (no max-subtraction; |E|/32 small, fp32-safe);
  - [V|1]^T @ P^T gives attention numerator + softmax denominator; the
    denominator row is reciprocal'd in place and broadcast across partitions
    with one SBUF->SBUF DMA;
  - AV, V-projection and output-projection matmuls are interleaved into the
    scores k-loop so the tensor stream fills the exp-bound gaps;
  - output projection accumulates head-pairs 0,1 into SBUF (ScalarE copies),
    then pairs 2,3 are added on VectorE and streamed out as bf16 partials.
"""

import sys
from contextlib import ExitStack

sys.path.insert(0, "/opt/trn_rl_repo")

import numpy as np

import concourse.bacc as bacc
import concourse.tile as tile
from concourse import mybir
from concourse.bass_utils import run_bass_kernel_spmd

EMBED = 1024
HEADS = 16
HEAD_DIM = 64
N_BATCH = 4
L = 1024
N_CORES = 8
HG = HEADS // 2          # heads per core
S = HG * HEAD_DIM        # per-core head-slice width (512)
KT = EMBED // 128        # k-tiles over the embed contraction dim (8)
MT = S // 128            # m-tiles over the head-slice dim (4)
F32 = mybir.dt.float32
BF16 = mybir.dt.bfloat16
MMD = BF16

_CACHED = {}


def _build(apply_mask: bool):
    nc = bacc.Bacc("TRN2", target_bir_lowering=False, debug=False,
                   num_devices=N_CORES)

    xqT = nc.dram_tensor("xqT", [EMBED, L], MMD, kind="ExternalInput").ap()
    xkT = nc.dram_tensor("xkT", [EMBED, L], MMD, kind="ExternalInput").ap()
    xvT = nc.dram_tensor("xvT", [EMBED, L], MMD, kind="ExternalInput").ap()
    wqT = nc.dram_tensor("wqT", [EMBED, S], MMD, kind="ExternalInput").ap()
    wkT = nc.dram_tensor("wkT", [EMBED, S], MMD, kind="ExternalInput").ap()
    wvT = nc.dram_tensor("wvT", [EMBED, S], MMD, kind="ExternalInput").ap()
    woT = nc.dram_tensor("woT", [S, EMBED], MMD, kind="ExternalInput").ap()
    bq_d = nc.dram_tensor("bq", [128, MT], F32, kind="ExternalInput").ap()
    bk_d = nc.dram_tensor("bk", [128, MT], F32, kind="ExternalInput").ap()
    bv_d = nc.dram_tensor("bv", [1, S], MMD, kind="ExternalInput").ap()
    ones_d = nc.dram_tensor("ones", [128, 128], MMD, kind="ExternalInput").ap()
    if apply_mask:
        mb_d = nc.dram_tensor("maskbT", [L, L], F32, kind="ExternalInput").ap()
    out_d = nc.dram_tensor("out_partial", [L, EMBED], MMD,
                           kind="ExternalOutput").ap()

    with tile.TileContext(nc) as tc, ExitStack() as ctx:
        sb = ctx.enter_context(tc.tile_pool(name="sb", bufs=2))
        ps = ctx.enter_context(tc.tile_pool(name="ps", bufs=2, space="PSUM"))
        dr = ctx.enter_context(tc.tile_pool(name="dr", bufs=2, space="DRAM"))
        p2 = ctx.enter_context(tc.tile_pool(name="p2", bufs=2))

        # constants / biases
        bq_sb = sb.tile([128, MT], F32, tag="bias")
        bk_sb = sb.tile([128, MT], F32, tag="bias")
        bv_sb = sb.tile([1, S], MMD, tag="bvrow")
        ones1 = sb.tile([1, 128], MMD, tag="ones1")
        nc.sync.dma_start(bq_sb[:], bq_d[:])
        nc.sync.dma_start(bk_sb[:], bk_d[:])
        nc.sync.dma_start(bv_sb[:], bv_d[:])
        nc.sync.dma_start(ones1[:], ones_d[0:1, :])

        # weight/x tiles (persist through their projection phase)
        p1_cm = tc.tile_pool(name="p1", bufs=2)
        p1 = p1_cm.__enter__()

        def load_chunk(src, width, tag, bufs, nm, c, eng):
            t = p1.tile([128, 4 * width], MMD, tag=tag, bufs=bufs,
                        name=f"{nm}{c}")
            eng.dma_start(
                t[:].rearrange("p (k l) -> p k l", k=4),
                src[c * 512:(c + 1) * 512, :].rearrange(
                    "(k p) l -> p k l", p=128))
            return [t[:, k * width:(k + 1) * width] for k in range(4)]

        def load_wx(name, wsrc, xsrc, xnm):
            wv_, xv_ = [], []
            for c in range(2):
                wv_ += load_chunk(wsrc, S, f"w_{name}", 2, f"w{name}", c,
                                  nc.sync)
                xv_ += load_chunk(xsrc, L, "x", 3, xnm, c, nc.gpsimd)
            return wv_, xv_

        # ---- k / q projections -> transposed layout [head_dim_slice, pos]
        def proj_T(x_tiles, w_tiles, bias_sb, out_tag):
            outs = []
            for m in range(MT):
                p = ps.tile([128, L], F32, tag="pa", bufs=2)
                for ch in range(2):
                    cs = slice(ch * 512, (ch + 1) * 512)
                    for k in range(KT):
                        nc.tensor.matmul(
                            p[:, cs],
                            (w_tiles[k][:, m * 128:(m + 1) * 128]),
                            (x_tiles[k][:, cs]),
                            start=(k == 0), stop=(k == KT - 1))
                o = sb.tile([128, L], MMD, tag=out_tag, bufs=MT)
                nc.scalar.activation(o[:], p[:],
                                     mybir.ActivationFunctionType.Identity,
                                     bias=bias_sb[:, m:m + 1])
                outs.append(o)
            return outs

        # PE clock warmup: dummy matmuls while the first DMAs land, so the
        # HAM un-throttles (1.2 -> 2.4 GHz) before real work starts.
        warm_ps = ps.tile([128, 512], F32, tag="pb", bufs=2)
        for i in range(8):
            nc.tensor.matmul(warm_ps[:, 0:512], (ones1[:]),
                             (bv_sb[0:1, 0:512]), start=True, stop=True)

        wk_t, xk_tiles = load_wx("k", wkT, xkT, "xk")
        kT_t = proj_T(xk_tiles, wk_t, bk_sb, "kT")
        wq_t, xq_tiles = load_wx("q", wqT, xqT, "xq")
        qT_t = proj_T(xq_tiles, wq_t, bq_sb, "qT")

        wo_t = []
        for c in range(MT):
            t = p2.tile([128, EMBED], MMD, tag="wo", bufs=MT, name=f"wo{c}")
            nc.sync.dma_start(t[:], woT[c * 128:(c + 1) * 128, :])
            wo_t.append(t)

        wv_t, xv_tiles = load_wx("v", wvT, xvT, "xv")

        if apply_mask:
            mb_t = []
            for k in range(KT):
                t = p2.tile([128, L], F32, tag="mb", bufs=KT)
                nc.sync.dma_start(t[:], mb_d[k * 128:(k + 1) * 128, :])
                mb_t.append(t)

        # normalized att-out^T pairs; SBUF accumulators for the out-projection
        xn_t = [sb.tile([128, L], MMD, tag="xn", bufs=MT, name=f"xn{i}")
                for i in range(MT)]
        out_acc = [sb.tile([128, EMBED], F32, tag="oacc", bufs=KT,
                           name=f"oacc{i}") for i in range(KT)]

        v_t = []

        def vproj_step(mp):
            # one pos-tile of the v projection: 8+1 matmuls + evacuation
            def step():
                p = ps.tile([128, S], F32, tag="pb", bufs=2, name=f"vp{mp}")
                for k in range(KT):
                    nc.tensor.matmul(p[:],
                                     (xv_tiles[k][:, mp * 128:(mp + 1) * 128]),
                                     (wv_t[k]), start=(k == 0), stop=False)
                nc.tensor.matmul(p[:], (ones1[:]), (bv_sb[:]),
                                 start=False, stop=True)
                vb = sb.tile([128, HG * 65], MMD, tag="vb", bufs=KT,
                             name=f"vb{mp}")
                vb3 = vb[:].rearrange("p (h d) -> p h d", h=HG)
                nc.sync.dma_start(
                    vb3[:, :, 64:65],
                    ones_d[:, 0:HG].rearrange("p (h d) -> p h d", d=1))
                nc.vector.tensor_copy(vb3[:, :, 0:64],
                                      p[:].rearrange("p (h d) -> p h d", h=HG))
                v_t.append(vb)
            return step

        def emit_scores_pair(m, steps):
            # heads 2m / 2m+1 stream concurrently in PE row-tiles T0/T8
            pts = {0: [], 1: []}
            for k in range(KT):
                if steps is not None:
                    steps[k]()
                e0 = ps.tile([128, L], F32, tag="pa", bufs=2,
                             name=f"e{2 * m}_{k}")
                e1 = ps.tile([128, L], F32, tag="pa", bufs=2,
                             name=f"e{2 * m + 1}_{k}")
                ks = slice(k * 128, (k + 1) * 128)
                for ch in range(2):
                    cs = slice(ch * 512, (ch + 1) * 512)
                    nc.tensor.matmul(e0[:, cs], (kT_t[m][0:64, ks]),
                                     (qT_t[m][0:64, cs]),
                                     start=True, stop=True)
                    nc.tensor.matmul(e1[:, cs], (kT_t[m][64:128, ks]),
                                     (qT_t[m][64:128, cs]),
                                     start=True, stop=True)
                for j, e in ((0, e0), (1, e1)):
                    h = 2 * m + j
                    pt = p2.tile([128, L], MMD, tag="pt", bufs=32,
                                 name=f"pt{h}_{k}")
                    if apply_mask:
                        es = p2.tile([128, L], F32, tag="es", bufs=2,
                                     name=f"es{h}_{k}")
                        nc.vector.tensor_add(es[:], e[:], mb_t[k][:])
                        nc.scalar.activation(pt[:], es[:],
                                             mybir.ActivationFunctionType.Exp)
                    else:
                        nc.scalar.activation(pt[:], e[:],
                                             mybir.ActivationFunctionType.Exp)
                    pts[j].append(pt)
            return pts

        def emit_normalize(h, m, j, o0, o1):
            den = dr.tile([1, L], F32, tag="den", name=f"den{h}")
            for ch, o in ((0, o0), (1, o1)):
                cs = slice(ch * 512, (ch + 1) * 512)
                den_row = p2.tile([65, 512], F32, tag="rcprow", bufs=2,
                                  name=f"dr{h}_{ch}")
                nc.vector.tensor_copy(den_row[64:65, :], o[64:65, :])
                nc.sync.dma_start(den[0:1, cs], den_row[64:65, :])
                den_b = p2.tile([64, 512], F32, tag="denb", bufs=2,
                                name=f"db{h}_{ch}")
                nc.sync.dma_start(den_b[:],
                                  den[0:1, cs].to_broadcast((64, 512)))
                rb = p2.tile([64, 512], F32, tag="rcpb", bufs=2,
                             name=f"rb{h}_{ch}")
                nc.vector.reciprocal_approx_fast(rb[:], den_b[:])
                if j == 0:
                    nc.vector.tensor_mul(xn_t[m][0:64, cs], o[0:64, :], rb[:])
                else:
                    xt = p2.tile([64, 512], MMD, tag="xtmp", bufs=2,
                                 name=f"xt{h}_{ch}")
                    nc.vector.tensor_mul(xt[:], o[0:64, :], rb[:])
                    nc.sync.dma_start(xn_t[m][64:128, cs], xt[:])

        def av_pair_steps(m, pts):
            # 8 step-closures, 4 AV matmuls each; normalize when a head ends
            h0, h1 = 2 * m, 2 * m + 1
            state = {}

            def mk(h, j, ch, klo, last):
                def step():
                    if klo == 0:
                        state[(h, ch)] = ps.tile(
                            [65, 512], F32, tag="po", bufs=2,
                            name=f"o{h}_{ch}")
                    o = state[(h, ch)]
                    cs = slice(ch * 512, (ch + 1) * 512)
                    for k in range(klo, klo + 4):
                        nc.tensor.matmul(o[:],
                                         (v_t[k][:, h * 65:(h + 1) * 65]),
                                         (pts[j][k][:, cs]),
                                         start=(k == 0), stop=(k == KT - 1),
                                         skip_group_check=True)
                    if last:
                        emit_normalize(h, m, j, state[(h, 0)], state[(h, 1)])
                return step

            return [mk(h0, 0, 0, 0, False), mk(h0, 0, 0, 4, False),
                    mk(h0, 0, 1, 0, False), mk(h0, 0, 1, 4, True),
                    mk(h1, 1, 0, 0, False), mk(h1, 1, 0, 4, False),
                    mk(h1, 1, 1, 0, False), mk(h1, 1, 1, 4, True)]

        def outproj_chunk_g0(qt, ec):
            qs = slice(qt * 128, (qt + 1) * 128)
            es_ = slice(ec * 512, (ec + 1) * 512)
            f = ps.tile([128, 512], F32, tag="pb", bufs=2,
                        name=f"f0_{qt}_{ec}")
            nc.tensor.matmul(f[:], (xn_t[0][:, qs]), (wo_t[0][:, es_]),
                             start=True, stop=False)
            nc.tensor.matmul(f[:], (xn_t[1][:, qs]), (wo_t[1][:, es_]),
                             start=False, stop=True)
            nc.scalar.copy(out_acc[qt][:, es_], f[:])

        def outproj_chunk_g1(qt, ec):
            qs = slice(qt * 128, (qt + 1) * 128)
            es_ = slice(ec * 512, (ec + 1) * 512)
            f = ps.tile([128, 512], F32, tag="pb", bufs=2,
                        name=f"f1_{qt}_{ec}")
            nc.tensor.matmul(f[:], (xn_t[2][:, qs]), (wo_t[2][:, es_]),
                             start=True, stop=False)
            nc.tensor.matmul(f[:], (xn_t[3][:, qs]), (wo_t[3][:, es_]),
                             start=False, stop=True)
            os_ = sb.tile([128, 512], MMD, tag="osb", bufs=2,
                          name=f"os{qt}_{ec}")
            nc.vector.tensor_add(os_[:], out_acc[qt][:, es_], f[:])
            nc.sync.dma_start(out_d[qs, es_], os_[:])

        def mix_steps(a_steps, chunks):
            # append 2 outproj chunks after each AV step
            out = []
            for i, st in enumerate(a_steps):
                def mk(st=st, i=i):
                    def step():
                        st()
                        outproj_chunk_g0(*chunks[2 * i])
                        outproj_chunk_g0(*chunks[2 * i + 1])
                    return step
                out.append(mk())
            return out

        g_chunks = [(qt, ec) for qt in range(KT) for ec in range(2)]

        INTERLEAVE = True
        if INTERLEAVE:
            pts0 = emit_scores_pair(0, [vproj_step(mp) for mp in range(KT)])
            pts1 = emit_scores_pair(1, av_pair_steps(0, pts0))
            pts2 = emit_scores_pair(2, av_pair_steps(1, pts1))
            pts3 = emit_scores_pair(3, mix_steps(av_pair_steps(2, pts2),
                                                 g_chunks))
            for st in av_pair_steps(3, pts3):
                st()
            for qt, ec in g_chunks:
                outproj_chunk_g1(qt, ec)
        else:
            pts0 = emit_scores_pair(0, None)
            for mp in range(KT):
                vproj_step(mp)()
            pts1 = emit_scores_pair(1, None)
            pts2 = emit_scores_pair(2, None)
            pts3 = emit_scores_pair(3, None)
            for pts_m, m in ((pts0, 0), (pts1, 1), (pts2, 2), (pts3, 3)):
                for st in av_pair_steps(m, pts_m):
                    st()
            for qt, ec in g_chunks:
                outproj_chunk_g0(qt, ec)
            for qt, ec in g_chunks:
                outproj_chunk_g1(qt, ec)

        p1_cm.__exit__(None, None, None)

    nc.compile()
    return nc


def make_in_maps(values, keys, queries, mask, Wv, bv, Wk, bk, Wq, bq, Wo, bo):
    values = np.asarray(values, dtype=np.float32)
    keys = np.asarray(keys, dtype=np.float32)
    queries = np.asarray(queries, dtype=np.float32)
    mask = np.asarray(mask)
    Wv, bv = np.asarray(Wv, np.float32), np.asarray(bv, np.float32)
    Wk, bk = np.asarray(Wk, np.float32), np.asarray(bk, np.float32)
    # fold the 1/sqrt(EMBED) softmax scale into the q projection
    Wq = np.asarray(Wq, np.float32) * (1.0 / 32.0)
    bq = np.asarray(bq, np.float32) * (1.0 / 32.0)
    Wo = np.asarray(Wo, np.float32)

    apply_mask = not bool(np.all(mask != 0))
    import ml_dtypes
    mmd_np = ml_dtypes.bfloat16

    def ct(a):
        return np.ascontiguousarray(np.asarray(a, dtype=np.float32))

    def cm(a):
        return np.ascontiguousarray(np.asarray(a).astype(mmd_np))

    in_maps = []
    for c in range(N_CORES):
        n, g = c // 2, c % 2
        sl = slice(g * S, (g + 1) * S)
        m = {
            "xqT": cm(queries[n].T),
            "xkT": cm(keys[n].T),
            "xvT": cm(values[n].T),
            "wqT": cm(Wq[sl, :].T),
            "wkT": cm(Wk[sl, :].T),
            "wvT": cm(Wv[sl, :].T),
            "woT": cm(Wo[:, sl].T),
            "bq": ct(bq[sl].reshape(MT, 128).T),
            "bk": ct(bk[sl].reshape(MT, 128).T),
            "bv": cm(bv[sl].reshape(1, S)),
            "ones": np.ones((128, 128), mmd_np),
        }
        if apply_mask:
            mb = np.where(mask[n, 0] == 0, np.float32(-1e18), np.float32(0.0))
            m["maskbT"] = ct(mb.T)
        in_maps.append(m)
    return in_maps, apply_mask


def kernel(values, keys, queries, mask, Wv, bv, Wk, bk, Wq, bq, Wo, bo):
    in_maps, apply_mask = make_in_maps(values, keys, queries, mask, Wv, bv,
                                       Wk, bk, Wq, bq, Wo, bo)
    if apply_mask not in _CACHED:
        _CACHED[apply_mask] = _build(apply_mask)
    nc = _CACHED[apply_mask]

    res = run_bass_kernel_spmd(nc, in_maps, list(range(N_CORES))).results
    bo = np.asarray(bo, np.float32)
    out = np.empty((N_BATCH, L, EMBED), dtype=np.float32)
    for n in range(N_BATCH):
        out[n] = (res[2 * n]["out_partial"].astype(np.float32)
                  + res[2 * n + 1]["out_partial"].astype(np.float32)
                  + bo[None, :])
    return out
